# revision 22
# baseline (speedup 1.0000x reference)
"""Causal multi-head attention block (B=16, S=1024, E=256, H=4, D=64) on 8
Trainium2 NeuronCores, data-parallel over batch (2 batches per core).

v2 design notes (vs baseline):
  - "natural-O" attention: P@V is computed with q on the output PARTITION dim
    (lhsT = P^T block [k,128q], rhs = V_ext [k, 65]), so PV pushes only 65
    free rows per (qc,kc) block instead of 512 — full PE utilization — and
    the softmax denominators (ones column of V_ext) land per-partition,
    where a cheap DVE reciprocal + broadcast multiply normalizes them.
  - bf16 operands everywhere on the PE (1 cycle/row at any width; f32r needs
    >=256-wide outputs for full rate). Total bf16 rounding ~0.5% rel << 2e-2.
  - bias algebra: K-bias is dropped entirely (adds a per-q constant to every
    score row -> softmax invariant); Q-bias is added during the Q^T PSUM
    evacuation as a per-partition DVE tensor_scalar; V-bias is folded into a
    combined output bias bc = b_v @ W_out + b_out applied as an extra
    rank-1 matmul pass (ones (x) bc_row) in the output projection.
  - Activation engine runs ONLY exp, in head-pair-merged [128,2,*] tiles
    (24 instructions/batch) to amortize the ~185ns/inst access overhead.
  - x^T via DMA xbar transpose (dma_start_transpose, bf16) - no PE/PSUM
    involvement; x is converted f32->bf16 once on Pool (SBUF->SBUF; note
    GPSIMD cannot touch PSUM on real HW, so all PSUM evacuation is DVE).
  - engine split: Act = exp only; DVE = all PSUM evacuation (Q^T+bias, K^T,
    V, Y+bias, O^T, normalize, tri-mask); Pool = x f32->bf16 converts and
    memsets; SP = all DMA issues; emission is software-pipelined across
    batches (next batch's load/QKV emitted inside this batch's attention).
"""
import sys

for p in ("/opt/trn_rl_repo",):
    if p not in sys.path:
        sys.path.insert(0, p)

import numpy as np

import concourse.bass as bass
import concourse.mybir as mybir
import concourse.tile as tile
from concourse.masks import make_identity
from concourse.vector_clock import ScopedClock

F32 = mybir.dt.float32
BF16 = mybir.dt.bfloat16
AF = mybir.ActivationFunctionType
ALU = mybir.AluOpType

N_CORES = 8
B, S, E, H, D = 16, 1024, 256, 4, 64
BPC = B // N_CORES          # batches per core
WAIT_CAP = 1


class TC(tile.TileContext):
    """TileContext workaround: this walrus build accepts at most one sync
    wait per instruction, so excess waits are peeled onto same-engine NOPs
    emitted immediately before the owning instruction (same semantics: the
    engine blocks on each in order)."""

    def _split_excess_waits(self, inst):
        si = inst.sync_info
        if si is None or len(si.on_wait) <= WAIT_CAP:
            return []
        waits = list(si.on_wait)
        kept, extra = waits[-WAIT_CAP:], waits[:-WAIT_CAP]
        nops = []
        for w in extra:
            nops.append(
                mybir.InstNoOp(
                    name=self.nc.get_next_instruction_name(),
                    engine=inst.engine,
                    sync_info=mybir.SyncInfo(on_wait=[w], on_update=[]),
                    ins=[],
                    outs=[],
                    bass_nofuse=True,
                )
            )
        inst.sync_info = mybir.SyncInfo(on_wait=kept, on_update=list(si.on_update))
        return nops

    def _add_instruction(self, inst):
        for n in self._split_excess_waits(inst):
            super()._add_instruction(n)
        super()._add_instruction(inst)

    def _drain_and_barrier(self, tick_clock, wait_clock):
        probe = self.nc.sync.nop(nofuse=True)
        wait_clock.add_sem_waits(probe.ins, ScopedClock({None: tick_clock.global_clock}))
        si = probe.ins.sync_info
        waits = list(si.on_wait) if si is not None else []
        probe.ins.sync_info = mybir.SyncInfo(on_wait=waits[:1], on_update=[])
        for w in waits[1:]:
            n2 = self.nc.sync.nop(nofuse=True)
            n2.ins.sync_info = mybir.SyncInfo(on_wait=[w], on_update=[])
        self.nc.sync.drain()
        self.nc.all_engine_barrier()
        assert self.sems is not None
        popped = self.nc._tile_sem_poison_stack.pop()
        assert popped is self._sem_poison
        self.nc.clear_and_free_semaphores(list(self.sems.allocated().values()))
        self.nc.all_engine_barrier()


def build_nc(reps: int = 1, **kw):
    nc = bass.Bass()
    x = nc.dram_tensor("x", [BPC, S, E], F32, kind="ExternalInput")
    w_qkv = nc.dram_tensor("W_qkv", [E, 3 * E], F32, kind="ExternalInput")
    b_qkv = nc.dram_tensor("b_qkv", [3 * E], F32, kind="ExternalInput")
    w_out = nc.dram_tensor("W_out", [E, E], F32, kind="ExternalInput")
    b_out = nc.dram_tensor("b_out", [E], F32, kind="ExternalInput")
    out = nc.dram_tensor("out", [BPC, S, E], F32, kind="ExternalOutput")

    with TC(nc) as tc:
        _emit(nc, tc, x, w_qkv, b_qkv, w_out, b_out, out, reps, **kw)
    return nc


def _emit(nc, tc, x, w_qkv, b_qkv, w_out, b_out, out, reps=1, xbar=True):
    import contextlib

    ctx = contextlib.ExitStack()
    with ctx:
        singles = ctx.enter_context(tc.tile_pool(name="singles", bufs=1))
        sb = ctx.enter_context(tc.tile_pool(name="sb", bufs=2))
        ps = ctx.enter_context(tc.tile_pool(name="ps", bufs=2, space="PSUM"))

        # ---------------- one-time setup ----------------
        # weights: DMA f32, convert to bf16 on DVE
        wraw = singles.tile([128, 2, 768], F32, tag="wraw")
        nc.sync.dma_start(out=wraw[:, 0, :], in_=w_qkv.ap()[0:128, :])
        nc.sync.dma_start(out=wraw[:, 1, :], in_=w_qkv.ap()[128:256, :])
        woraw = singles.tile([128, 2, 256], F32, tag="woraw")
        nc.sync.dma_start(out=woraw[:, 0, :], in_=w_out.ap()[0:128, :])
        nc.sync.dma_start(out=woraw[:, 1, :], in_=w_out.ap()[128:256, :])
        wrb = singles.tile([128, 2, 768], BF16, tag="wrb")
        nc.vector.tensor_copy(wrb, wraw)
        worb = singles.tile([128, 2, 256], BF16, tag="worb")
        nc.vector.tensor_copy(worb, woraw)

        # biases
        bqk_col = singles.tile([128, 4], F32, tag="bqk_col")
        nc.sync.dma_start(
            out=bqk_col, in_=b_qkv.ap()[0:512].rearrange("(c p) -> p c", p=128)
        )
        bv_raw = singles.tile([128, 2], F32, tag="bv_raw")
        nc.sync.dma_start(
            out=bv_raw, in_=b_qkv.ap()[512:768].rearrange("(c p) -> p c", p=128)
        )
        bv_bf = singles.tile([128, 2], BF16, tag="bv_bf")
        nc.vector.tensor_copy(bv_bf, bv_raw)
        bout_row = singles.tile([1, 256], F32, tag="bout_row")
        nc.sync.dma_start(out=bout_row, in_=b_out.ap()[None, :])

        ones_bf = singles.tile([1, 128], BF16, tag="ones_bf")
        nc.vector.memset(ones_bf, 1.0)

        # identity (bf16) for PE transposes of O
        ident_bf = singles.tile([128, 128], BF16, tag="ident_bf")
        make_identity(nc, ident_bf)

        # tri mask [128,128] bf16: tri[p, y] = 1.0 if y >= p else 0.0
        tri_bf = singles.tile([128, 128], BF16, tag="tri_bf")
        nc.gpsimd.memset(tri_bf, 1.0)
        nc.gpsimd.affine_select(
            out=tri_bf,
            in_=tri_bf,
            compare_op=ALU.is_ge,
            fill=0.0,
            base=0,
            pattern=[[1, 128]],
            channel_multiplier=-1,
        )

        # combined output bias bc = b_v @ W_out + b_out, broadcast [128,2,256]
        ps_bc = ps.tile([128, 2, 256], F32, tag="PD", name="ps_bc")
        for j in range(2):
            nc.tensor.matmul(
                ps_bc[0:1, 0, :], bv_bf[:, j : j + 1], worb[:, j, :],
                start=(j == 0), stop=(j == 1),
            )
        bc_row = singles.tile([1, 256], BF16, tag="bc_row")
        nc.vector.tensor_add(bc_row, ps_bc[0:1, 0, :], bout_row)
        ps_bc2 = ps.tile([128, 2, 256], F32, tag="PD", name="ps_bc2")
        nc.tensor.matmul(ps_bc2[:, 0, :], ones_bf, bc_row, start=True, stop=True)
        bc2 = singles.tile([128, 2, 256], F32, tag="bc2")
        for j in range(2):
            nc.vector.tensor_copy(bc2[:, j, :], ps_bc2[:, 0, :])

        # ---------------- per-batch pipeline (software-pipelined) ----------------
        def stage_a(b):
            """Load x, convert to bf16, xT via DMA xbar transpose."""
            xing = sb.tile([128, 8, 256], F32, tag="xing", bufs=2)
            for g in range(2):
                nc.sync.dma_start(
                    out=xing[:, 4 * g : 4 * g + 4, :],
                    in_=x.ap()[b, g * 512 : (g + 1) * 512, :].rearrange(
                        "(j p) e -> p j e", j=4
                    ),
                )
            xbf = sb.tile([128, 8, 256], BF16, tag="xbf", bufs=2)
            for g in range(2):
                nc.gpsimd.tensor_copy(
                    xbf[:, 4 * g : 4 * g + 4, :], xing[:, 4 * g : 4 * g + 4, :]
                )
            xT = sb.tile([128, 2, 1024], BF16, tag="xT", bufs=2)
            for sj in range(8):
                nc.sync.dma_start_transpose(
                    out=xT[:, :, sj * 128 : (sj + 1) * 128],
                    in_=xbf[:, sj, :],
                )
            return xT

        def stage_b(xT):
            """QKV projections. qkT m=0,1: Q^T (+bias); m=2,3: K^T. V_ext."""
            qkT = [
                sb.tile([128, 1024], BF16, tag=f"qkT{m}", name=f"qkT{m}", bufs=2)
                for m in range(4)
            ]
            for m in range(4):
                for qb in range(2):
                    ps_qk = ps.tile(
                        [128, 2, 256], F32, tag="PD", bufs=2, name=f"ps_qk{m}{qb}"
                    )
                    flat = ps_qk.rearrange("p a b -> p (a b)")
                    for ec in range(2):
                        nc.tensor.matmul(
                            flat,
                            wrb[:, ec, m * 128 : (m + 1) * 128],
                            xT[:, ec, qb * 512 : (qb + 1) * 512],
                            start=(ec == 0),
                            stop=(ec == 1),
                        )
                    if m < 2:
                        nc.vector.tensor_scalar_add(
                            qkT[m][:, qb * 512 : (qb + 1) * 512],
                            flat,
                            bqk_col[:, m : m + 1],
                        )
                    else:
                        nc.vector.tensor_copy(
                            qkT[m][:, qb * 512 : (qb + 1) * 512], flat
                        )
            vext = sb.tile([128, 8, 4, 65], BF16, tag="vext", bufs=2)
            nc.gpsimd.memset(vext[:, :, :, 64:65], 1.0)
            for sc2 in range(4):
                ps_v = ps.tile([128, 2, 256], F32, tag="PD", bufs=2, name=f"ps_v{sc2}")
                for jj in range(2):
                    for ec in range(2):
                        nc.tensor.matmul(
                            ps_v[:, jj, :],
                            xT[:, ec, (2 * sc2 + jj) * 128 : (2 * sc2 + jj + 1) * 128],
                            wrb[:, ec, 512:768],
                            start=(ec == 0),
                            stop=(ec == 1),
                        )
                nc.vector.tensor_copy(
                    vext[:, 2 * sc2 : 2 * sc2 + 2, :, 0:64],
                    ps_v.rearrange("p j (h d) -> p j h d", d=64),
                )
            return qkT, vext

        class St:
            pass

        def attn_open(b):
            st = St()
            st.b = b
            # packed per-strip P^T: j0 @ [0:512], j3 @ [512:640],
            # j1 @ [640:1024], j2 @ [1024:1280]  (block j valid y in [j*128,512))
            st.pT_diag = [
                sb.tile([128, 4, 1280], BF16, tag=f"pTd{qb}", name=f"pTd{qb}", bufs=2)
                for qb in range(2)
            ]
            st.pT_full = sb.tile([128, 4, 4, 512], BF16, tag="pTf", name="pTf", bufs=2)
            st.osb = sb.tile([128, 8, 4, 64], BF16, tag="osb", bufs=2)
            st.ohT = sb.tile([128, 2, 1024], BF16, tag="ohT", bufs=2)
            return st

        # packed offsets: PT_OFF[j] = column of block j's first valid y
        PT_OFF = [0, 640, 1024, 512]

        def diag_part(st, qb, t):
            # t=0: j0 (psum [0:512] -> pT [0:512])
            # t=1: j3 (psum [0:128] -> pT [512:640]) + j1 ([128:512] -> [640:1024])
            tile_js = [[(0, 0)], [(3, 0), (1, 128)], [(2, 256)]][t]
            lo = min(c for _, c in tile_js)
            hi = max(c + 512 - j * 128 for j, c in tile_js)
            po = PT_OFF[tile_js[0][0]] - lo
            for hp in range(2):
                ps_s = ps.tile(
                    [128, 2, 512], F32, tag="P2", bufs=2, name=f"ps_d{qb}{t}{hp}"
                )
                for j, c in tile_js:
                    y0 = j * 128
                    kc = 4 * qb + j
                    for h2 in range(2):
                        hr = 64 * h2
                        nc.tensor.matmul(
                            ps_s[:, h2, c : c + 512 - y0],
                            st.qkT[2 + hp][hr : hr + 64, kc * 128 : (kc + 1) * 128],
                            st.qkT[hp][hr : hr + 64, qb * 512 + y0 : (qb + 1) * 512],
                            start=True,
                            stop=True,
                        )
                nc.scalar.activation(
                    out=st.pT_diag[qb][:, 2 * hp : 2 * hp + 2, po + lo : po + hi],
                    in_=ps_s[:, :, lo:hi],
                    func=AF.Exp,
                    scale=0.125,
                )
            # zero the acausal triangles (leading 128-strip of each block)
            for j, _ in tile_js:
                strip = st.pT_diag[qb][:, :, PT_OFF[j] : PT_OFF[j] + 128]
                nc.vector.tensor_mul(
                    strip, strip, tri_bf.unsqueeze(1).broadcast_to([128, 4, 128])
                )

        def full_kc(st, kc):
            # qb=1 strip, kc in 0..3: full blocks
            for hp in range(2):
                ps_s = ps.tile(
                    [128, 2, 512], F32, tag="P2", bufs=2, name=f"ps_f{kc}{hp}"
                )
                for h2 in range(2):
                    hr = 64 * h2
                    nc.tensor.matmul(
                        ps_s[:, h2, :],
                        st.qkT[2 + hp][hr : hr + 64, kc * 128 : (kc + 1) * 128],
                        st.qkT[hp][hr : hr + 64, 512:1024],
                        start=True,
                        stop=True,
                    )
                nc.scalar.activation(
                    out=st.pT_full[:, kc, 2 * hp : 2 * hp + 2, :],
                    in_=ps_s,
                    func=AF.Exp,
                    scale=0.125,
                )

        def pv(st, qc):
            # O[128q, h, d+1] accumulated over kc = 0..qc
            qb, jq = qc // 4, qc % 4
            ps_o = ps.tile([128, 4, 65], F32, tag="PSV", bufs=2, name=f"ps_o{qc}")
            for h in range(H):
                for kc in range(qc + 1):
                    if kc < 4 * qb:  # full block (qb=1 only)
                        blk = st.pT_full[:, kc, h, jq * 128 : (jq + 1) * 128]
                    else:
                        j = kc - 4 * qb
                        c = PT_OFF[j] + (jq - j) * 128
                        blk = st.pT_diag[qb][:, h, c : c + 128]
                    nc.tensor.matmul(
                        ps_o[:, h, :],
                        blk,
                        st.vext[:, kc, h, :],
                        start=(kc == 0),
                        stop=(kc == qc),
                    )
            recip = sb.tile([128, 4], F32, tag="recip", bufs=3, name=f"rc{qc}")
            nc.vector.reciprocal(recip, ps_o[:, :, 64:65])
            nc.vector.tensor_mul(
                st.osb[:, qc, :, :],
                ps_o[:, :, 0:64],
                recip.unsqueeze(2).broadcast_to([128, 4, 64]),
            )

        def o_transpose(st, qg):
            # O^T for q-chunks 4qg..4qg+3 via DMA xbar (SBUF->SBUF, bf16):
            # no PE/PSUM/DVE involvement at all.
            for qi in range(4):
                qc = 4 * qg + qi
                nc.sync.dma_start_transpose(
                    out=st.ohT[:, :, qc * 128 : (qc + 1) * 128],
                    in_=st.osb[:, qc, :, :].rearrange("p a b -> p (a b)"),
                )

        def proj(st, sg):
            # Y for s-chunks 2sg, 2sg+1 (combined bias added in Pool evac)
            ps_y = ps.tile([128, 2, 256], F32, tag="PD", bufs=2, name=f"ps_y{sg}")
            for jj in range(2):
                sc = 2 * sg + jj
                for ec in range(2):
                    nc.tensor.matmul(
                        ps_y[:, jj, :],
                        st.ohT[:, ec, sc * 128 : (sc + 1) * 128],
                        worb[:, ec, :],
                        start=(ec == 0),
                        stop=(ec == 1),
                    )
            ysb = sb.tile([128, 2, 256], F32, tag="ysb", bufs=3, name=f"y{sg}")
            nc.vector.tensor_add(ysb, ps_y, bc2)
            nc.sync.dma_start(
                out=out.ap()[st.b, sg * 256 : (sg + 1) * 256, :].rearrange(
                    "(j p) e -> p j e", j=2
                ),
                in_=ysb,
            )

        # Software-pipelined emission: batch i+1's load/QKV stages are emitted
        # inside batch i's attention so the Act engine never drains at batch
        # boundaries, and each PV lags its diag strip by one step so the PE
        # does not stall on the freshly-emitted exp.
        bs = [b for _ in range(reps) for b in range(BPC)]

        def head(stt):
            # first two diag tiles -- emitted inside the previous batch so
            # the Act engine rolls straight across the batch boundary
            diag_part(stt, 0, 0)
            diag_part(stt, 0, 1)

        st = attn_open(bs[0])
        st.xT = stage_a(bs[0])
        st.qkT, st.vext = stage_b(st.xT)
        head(st)
        prev = None  # batch with deferred tail (OT1 + proj2/3)
        for i, b in enumerate(bs):
            nxt = None
            diag_part(st, 0, 2)
            pv(st, 0)
            if prev is not None:  # tail of previous batch inside Act-paced diag
                proj(prev, 2)
                proj(prev, 3)
                prev = None
            pv(st, 1)
            full_kc(st, 0)
            pv(st, 2)
            full_kc(st, 1)
            pv(st, 3)
            o_transpose(st, 0)
            proj(st, 0)
            proj(st, 1)
            if i + 1 < len(bs):
                nxt = attn_open(bs[i + 1])
                nxt.xT = stage_a(bs[i + 1])
            full_kc(st, 2)
            full_kc(st, 3)
            if nxt is not None:
                nxt.qkT, nxt.vext = stage_b(nxt.xT)
            diag_part(st, 1, 0)
            diag_part(st, 1, 1)
            pv(st, 4)
            diag_part(st, 1, 2)
            if nxt is not None:
                diag_part(nxt, 0, 0)
            pv(st, 5)
            if nxt is not None:
                diag_part(nxt, 0, 1)
            pv(st, 6)
            pv(st, 7)
            o_transpose(st, 1)
            prev = st
            if nxt is not None:
                st = nxt
        # drain the last batch's tail
        proj(prev, 2)
        proj(prev, 3)

# ---------------- host-side runner ----------------
_RUNNER = {}


class _Runner:
    """Compile once, run many: replicates bass2jax.run_bass_via_pjrt's
    shard_map-over-8-devices path but caches the jitted callable."""

    def __init__(self, reps: int = 1):
        import jax
        from jax.sharding import Mesh, PartitionSpec
        from jax.experimental.shard_map import shard_map
        from concourse import bass2jax

        bass2jax.install_neuronx_cc_hook()
        nc = build_nc(reps)
        self.nc = nc

        partition_name = nc.partition_id_tensor.name if nc.partition_id_tensor else None
        in_names, out_names, out_avals, zero_outs = [], [], [], []
        for alloc in nc.m.functions[0].allocations:
            if not isinstance(alloc, mybir.MemoryLocationSet):
                continue
            name = alloc.memorylocations[0].name
            if alloc.kind == "ExternalInput":
                if name != partition_name:
                    in_names.append(name)
            elif alloc.kind == "ExternalOutput":
                out_names.append(name)
                shape = tuple(alloc.tensor_shape)
                dtype = mybir.dt.np(alloc.dtype)
                out_avals.append(jax.core.ShapedArray(shape, dtype))
                zero_outs.append(np.zeros(shape, dtype))
        self.n_params = len(in_names)
        n_outs = len(out_avals)
        self.in_names = list(in_names)
        self.out_names = out_names
        self.out_avals = out_avals
        self.zero_outs = zero_outs
        in_names = in_names + out_names
        if partition_name is not None:
            in_names.append(partition_name)

        def _body(*args):
            operands = list(args)
            if partition_name is not None:
                operands.append(bass2jax.partition_id_tensor())
            outs = bass2jax._bass_exec_p.bind(
                *operands,
                out_avals=tuple(out_avals),
                in_names=tuple(in_names),
                out_names=tuple(out_names),
                lowering_input_output_aliases=(),
                sim_require_finite=True,
                sim_require_nnan=True,
                nc=nc,
            )
            return tuple(outs)

        devices = jax.devices()[:N_CORES]
        mesh = Mesh(np.asarray(devices), ("core",))
        in_specs = (PartitionSpec("core"),) * (self.n_params + n_outs)
        out_specs = (PartitionSpec("core"),) * len(out_names)
        self.fn = jax.jit(
            shard_map(
                _body, mesh=mesh, in_specs=in_specs, out_specs=out_specs, check_rep=False
            ),
            donate_argnums=tuple(range(self.n_params, self.n_params + n_outs)),
            keep_unused=True,
        )

    def concat_inputs(self, in_maps):
        return [
            np.concatenate([np.asarray(m[name]) for m in in_maps], axis=0)
            for name in self.in_names
        ]

    def run_async(self, concat_in):
        concat_zeros = [
            np.zeros((N_CORES * z.shape[0], *z.shape[1:]), z.dtype)
            for z in self.zero_outs
        ]
        return self.fn(*concat_in, *concat_zeros)

    def run(self, in_maps):
        out_arrs = self.run_async(self.concat_inputs(in_maps))
        return [
            {
                name: np.asarray(out_arrs[i]).reshape(
                    N_CORES, *self.out_avals[i].shape
                )[c]
                for i, name in enumerate(self.out_names)
            }
            for c in range(N_CORES)
        ]


def _get_runner(reps: int = 1):
    if reps not in _RUNNER:
        _RUNNER[reps] = _Runner(reps)
    return _RUNNER[reps]


def kernel(x, W_qkv, b_qkv, W_out, b_out):
    x = np.ascontiguousarray(np.asarray(x, dtype=np.float32))
    in_maps = [
        {
            "x": x[c * BPC : (c + 1) * BPC],
            "W_qkv": np.asarray(W_qkv, np.float32),
            "b_qkv": np.asarray(b_qkv, np.float32),
            "W_out": np.asarray(W_out, np.float32),
            "b_out": np.asarray(b_out, np.float32),
        }
        for c in range(N_CORES)
    ]
    results = _get_runner().run(in_maps)
    return np.concatenate([r["out"] for r in results], axis=0)
